# revision 1
# baseline (speedup 1.0000x reference)
"""DKEPooling Trainium2 kernel.

Per-graph pipeline (d=256, n=512 nodes/graph):
  f = feat + 0.01*noise
  C' = f^T f - colsum(f)^T colssum(f)/n          (= (n-1)*cov, Gram + rank-1 PSUM trick)
  A  = C'/tr(C')
  Newton-Schulz (5 iter) reformulated via the commuting-polynomial invariant
  T_k := A Z_k^2:  T_{k+1} = 0.25 T_k (3I - T_k)^2   -> only 6 d^3 matmuls/graph
  (A^2, then 2 per T-step), followed by an 8-matvec tail applied to the mean
  (all remaining NS factors are applied vector-wise, never materialized).

Sharding: data-parallel over graphs. 8 cores x 16 graphs; no cross-core comm.
"""
import numpy as np

import concourse.bacc as bacc
import concourse.bass as bass
import concourse.mybir as mybir
import concourse.tile as tile
from concourse.bass_utils import run_bass_kernel_spmd

F32 = mybir.dt.float32
BF16 = mybir.dt.bfloat16
F32R = mybir.dt.float32r
ALU = mybir.AluOpType
ACTF = mybir.ActivationFunctionType

N_CORES = 8
D = 256
NPG = 512
B_TOTAL = 128
B_CORE = B_TOTAL // N_CORES      # 16 graphs per core
ROWS_CORE = B_CORE * NPG         # 8192 feat rows per core
W = 4                            # graphs per tail wave
N_WAVES = B_CORE // W

# const tensor layout (f32 [128, 772]):
#   [:, 0:256]   = [3I | 0]   (3I block for row-chunk 0)
#   [:, 256:512] = [0 | 3I]   (3I block for row-chunk 1)
#   [:, 512:640] = I128
#   [:, 640]     = ones column
#   [0, 641:769] = ones row
CST_COLS = 772


def _const_arrays():
    import ml_dtypes
    cst = np.zeros((128, CST_COLS), np.float32)
    eye = np.eye(128, dtype=np.float32)
    cst[:, 0:128] = 3.0 * eye
    cst[:, 384:512] = 3.0 * eye
    cst[:, 512:640] = eye
    cst[:, 640] = 1.0
    cst[0, 641:769] = 1.0
    cstb = np.ones((128, 1), ml_dtypes.bfloat16)
    cstr = np.eye(W, dtype=np.float32)
    return cst, cstb, cstr


def _r(ap):
    return ap.bitcast(F32R)


def build_module():
    nc = bacc.Bacc(None, target_bir_lowering=False)
    feat_d = nc.declare_dram_parameter("feat", [ROWS_CORE, D], F32, isOutput=False)
    noise_d = nc.declare_dram_parameter("noise", [ROWS_CORE, D], F32, isOutput=False)
    cst_d = nc.declare_dram_parameter("cst", [128, CST_COLS], F32, isOutput=False)
    cstb_d = nc.declare_dram_parameter("cstb", [128, 1], BF16, isOutput=False)
    cstr_d = nc.declare_dram_parameter("cstr", [W, W], F32R, isOutput=False)
    out_d = nc.declare_dram_parameter("out", [B_CORE, D], F32, isOutput=True)

    with tile.TileContext(nc) as tc:
        _build_tile(tc, nc, feat_d, noise_d, cst_d, cstb_d, cstr_d, out_d)
    nc.compile()
    return nc


def _build_tile(tc, nc, feat_d, noise_d, cst_d, cstb_d, cstr_d, out_d):
    import contextlib
    ctx = contextlib.ExitStack()
    with ctx:
        stage_p = ctx.enter_context(tc.tile_pool(name="stage", bufs=5))
        g_p = ctx.enter_context(tc.tile_pool(name="gp", bufs=6))
        mats_p = ctx.enter_context(tc.tile_pool(name="mats", bufs=7))
        chain_p = ctx.enter_context(tc.tile_pool(name="chain", bufs=3))
        small_p = ctx.enter_context(tc.tile_pool(name="small", bufs=6))
        rows_p = ctx.enter_context(tc.tile_pool(name="rows", bufs=3))
        tail_p = ctx.enter_context(tc.tile_pool(name="tailp", bufs=3))
        cst_p = ctx.enter_context(tc.tile_pool(name="cstp", bufs=1))
        psG = ctx.enter_context(tc.tile_pool(name="psG", bufs=3, space="PSUM"))
        psS = ctx.enter_context(tc.tile_pool(name="psS", bufs=1, space="PSUM"))
        psUR = ctx.enter_context(tc.tile_pool(name="psUR", bufs=2, space="PSUM"))
        psT = ctx.enter_context(tc.tile_pool(name="psT", bufs=2, space="PSUM"))

        cst = cst_p.tile([128, CST_COLS], F32, tag="cst", name="cst_sb")
        nc.gpsimd.dma_start(cst, cst_d[:, :])
        onesb = cst_p.tile([128, 1], BF16, tag="onesb", name="onesb_sb")
        nc.gpsimd.dma_start(onesb, cstb_d[:, :])
        IWr = cst_p.tile([W, W], F32R, tag="iwr", name="iwr_sb")
        nc.gpsimd.dma_start(IWr, cstr_d[:, :])

        def c3I(m):
            return cst[:, 256 * m:256 * (m + 1)]

        I128 = cst[:, 512:640]
        ones_col = cst[:, 640:641]
        ones_row = cst[0:1, 641:769]

        def phase_a(g, V0ROWS, b):
            """Load graph g, compute A and T1..T3; returns dict of kept tiles."""
            # One big [128, 4*256] tile per tensor: the graph's 512 rows as 4
            # row-chunks side by side in the free dim; a single SWDGE DMA each
            # (HWDGE fans one transfer across several queue semaphores, which
            # overflows the DVE consumers' wait slots).
            ft = stage_p.tile([128, 4 * D], F32, tag="ft", name=f"ft_{g}")
            nc.gpsimd.dma_start(
                ft, feat_d[g * NPG:(g + 1) * NPG, :].rearrange("(c p) d -> p c d", p=128))
            nz = stage_p.tile([128, 4 * D], F32, tag="nz", name=f"nz_{g}")
            nc.gpsimd.dma_start(
                nz, noise_d[g * NPG:(g + 1) * NPG, :].rearrange("(c p) d -> p c d", p=128))
            gb = g_p.tile([128, 4 * D], BF16, tag="g", name=f"g_{g}")
            # f = (noise * 0.01) + feat, rounded to bf16 for the Gram.
            # Keep the DVE arithmetic in plain f32 (in-place, standard ISA
            # structs with enough sync slots) and convert to bf16 on ACT;
            # dtype-converting DVE ops lower to custom ucode with too few
            # sync-wait slots for walrus.
            nc.vector.scalar_tensor_tensor(gb, nz, 0.01, ft, ALU.mult, ALU.add)
            gt = [gb[:, k * D:(k + 1) * D] for k in range(4)]

            # Gram into PSUM: G_m = sum_k g_k[:, m*128:...].T @ g_k   (stop on corr MM)
            G = [psG.tile([128, D], F32, tag="G", name=f"G{m}_{g}") for m in range(2)]
            for k in range(4):
                for m in range(2):
                    nc.tensor.matmul(G[m], gt[k][:, m * 128:(m + 1) * 128], gt[k],
                                     start=(k == 0), stop=False)
            # column sums s = ones^T g
            s_ps = psS.tile([1, D], F32, tag="ps_small", name=f"s_{g}")
            for k in range(4):
                nc.tensor.matmul(s_ps, onesb, gt[k], start=(k == 0), stop=(k == 3))
            srow = small_p.tile([1, D], BF16, tag="srow", name=f"srow_{g}")
            nc.scalar.copy(srow, s_ps)
            srow_n = small_p.tile([1, D], BF16, tag="srow_n", name=f"srown_{g}")
            nc.vector.tensor_scalar_mul(srow_n, srow, -1.0 / NPG)

            # rank-1 mean correction accumulated into the Gram PSUM:
            # C' = G - s^T s / n
            for m in range(2):
                nc.tensor.matmul(G[m], srow_n[0:1, m * 128:(m + 1) * 128], srow,
                                 start=False, stop=True)

            # Evacuate C' from PSUM via ACT (DVE reads of PSUM crash the
            # exec unit on this runtime; ACT reads are fine).
            Gc = []
            for m in range(2):
                gc = chain_p.tile([128, D], F32, tag=f"Gc{m}", name=f"Gc{m}_{g}")
                nc.scalar.copy(gc, G[m])
                Gc.append(gc)
            # trace via diag mask + GPSIMD partition all-reduce (the tiny
            # fp32 PE matmuls this used before crash the exec unit)
            scr = small_p.tile([128, 128], F32, tag="scr", name=f"scr_{g}")
            dg = small_p.tile([128, 2], F32, tag="dg", name=f"dg_{g}")
            for m in range(2):
                nc.vector.scalar_tensor_tensor(scr, Gc[m][:, m * 128:(m + 1) * 128],
                                               1.0, I128, ALU.mult, ALU.mult,
                                               accum_out=dg[:, m:m + 1])
            import concourse.bass_isa as bass_isa
            dgs = small_p.tile([128, 1], F32, tag="dgs", name=f"dgs_{g}")
            nc.vector.tensor_add(dgs, dg[:, 0:1], dg[:, 1:2])
            trc = small_p.tile([128, 1], F32, tag="trc", name=f"trc_{g}")
            nc.gpsimd.partition_all_reduce(trc, dgs, 128, bass_isa.ReduceOp.add)
            rcpb = small_p.tile([128, 1], F32, tag="rcpb", name=f"rcpb_{g}")
            nc.vector.reciprocal(rcpb, trc)
            sq = small_p.tile([1, 1], F32, tag="sq", name=f"sq_{g}")
            nc.scalar.activation(sq, trc[0:1, 0:1], ACTF.Sqrt, scale=1.0 / (NPG - 1))
            cb = small_p.tile([1, 1], F32, tag="cb", name=f"cb_{g}")
            nc.vector.tensor_scalar_mul(cb, sq, 0.03125 / NPG)
            # v0 row for the tail: mean scaled by all folded constants.
            # Computed at partition 0, DMA'd into row b of V0ROWS (compute
            # engines cannot write non-32-aligned partition bases).
            v0r = small_p.tile([1, D], F32R, tag="v0r", name=f"v0r_{g}")
            nc.scalar.activation(v0r, s_ps, ACTF.Copy, scale=cb)
            nc.sync.dma_start(V0ROWS[b:b + 1, :], v0r)

            A = []
            for m in range(2):
                Am = mats_p.tile([128, D], F32R, tag=f"A{m}", name=f"A{m}_{g}")
                nc.vector.tensor_scalar_mul(Am, Gc[m], rcpb)
                A.append(Am)

            def mm256(tag, L, R, dst_pool, dst_tag):
                dst = [dst_pool.tile([128, D], F32, tag=dst_tag, name=f"{tag}{m}_{g}")
                       for m in range(2)]
                for m in range(2):
                    for k in range(2):
                        nc.tensor.matmul(dst[m], L[k][:, m * 128:(m + 1) * 128],
                                         R[k], start=(k == 0), stop=(k == 1))
                return dst

            # T-chain: A2 -> T1 -> T2 -> T3 (2 matmuls per step after A2)
            A2 = mm256("A2", A, A, psG, "G")
            W1 = []
            V0 = []
            for m in range(2):
                a2c = chain_p.tile([128, D], F32, tag=f"A2c{m}", name=f"A2c{m}_{g}")
                nc.scalar.copy(a2c, A2[m])
                w1 = chain_p.tile([128, D], F32R, tag=f"W1{m}", name=f"W1{m}_{g}")
                nc.vector.scalar_tensor_tensor(w1, A[m], 3.0, a2c, ALU.mult, ALU.subtract)
                W1.append(w1)
                v0 = chain_p.tile([128, D], F32R, tag=f"V0{m}", name=f"V0{m}_{g}")
                nc.vector.scalar_tensor_tensor(v0, A[m], -1.0, c3I(m), ALU.mult, ALU.add)
                V0.append(v0)
            P = mm256("P", W1, V0, psG, "G")
            T1 = []
            V1 = []
            for m in range(2):
                t1 = mats_p.tile([128, D], F32R, tag=f"T1{m}", name=f"T1{m}_{g}")
                nc.scalar.mul(t1, P[m], 0.25)
                T1.append(t1)
                v1 = chain_p.tile([128, D], F32R, tag=f"V1{m}", name=f"V1{m}_{g}")
                nc.vector.scalar_tensor_tensor(v1, t1, -1.0, c3I(m), ALU.mult, ALU.add)
                V1.append(v1)
            Q = mm256("Q", T1, V1, psG, "G")
            Qb = []
            for m in range(2):
                qb = chain_p.tile([128, D], F32R, tag=f"Qb{m}", name=f"Qb{m}_{g}")
                nc.scalar.copy(qb, Q[m])
                Qb.append(qb)
            R = mm256("R", Qb, V1, psG, "G")
            T2 = []
            V2 = []
            for m in range(2):
                t2 = mats_p.tile([128, D], F32R, tag=f"T2{m}", name=f"T2{m}_{g}")
                nc.scalar.mul(t2, R[m], 0.25)
                T2.append(t2)
                v2 = chain_p.tile([128, D], F32R, tag=f"V2{m}", name=f"V2{m}_{g}")
                nc.vector.scalar_tensor_tensor(v2, t2, -1.0, c3I(m), ALU.mult, ALU.add)
                V2.append(v2)
            S = mm256("S", T2, V2, psG, "G")
            Sb = []
            for m in range(2):
                sb_ = chain_p.tile([128, D], F32R, tag=f"Sb{m}", name=f"Sb{m}_{g}")
                nc.scalar.copy(sb_, S[m])
                Sb.append(sb_)
            U = mm256("U", Sb, V2, psG, "G")
            T3 = []
            for m in range(2):
                t3 = mats_p.tile([128, D], F32R, tag=f"T3{m}", name=f"T3{m}_{g}")
                nc.scalar.mul(t3, U[m], 0.25)
                T3.append(t3)
            return {"A": A, "T1": T1, "T2": T2, "T3": T3}

        def matvec_step(si, wave, cur, mats, kind, v0c=None):
            """One tail step for all W graphs: u = X @ v (row-form + transpose back).

            Per-graph u rows land in PSUM at 32-aligned partitions (legal PE
            column-group bases), then a strided DMA gathers them to packed rows.
            Returns next v column tiles [128, W] x2."""
            xkey = {0: "T3", 1: "T3", 2: "T3", 3: "T3", 4: "T2", 5: "T1", 6: "A", 7: "A"}[si]
            usb = rows_p.tile([W, D], F32R, tag="usb", name=f"usb_{wave}_{si}")
            for b in range(W):
                X = mats[b][xkey]
                ur = psUR.tile([1, D], F32, tag="ur", name=f"ur_{wave}_{si}_{b}")
                for k in range(2):
                    nc.tensor.matmul(ur, cur[k][:, b:b + 1], X[k],
                                     start=(k == 0), stop=(k == 1))
                # PE can only write PSUM at base partition 0 here, and compute
                # engines cannot write partition b directly: copy to a
                # partition-0 row, then DMA-scatter into the packed row tile.
                us = small_p.tile([1, D], F32R, tag="us", name=f"us_{wave}_{si}_{b}")
                nc.scalar.copy(us, ur)
                if kind == "final":
                    nc.sync.dma_start(out_d[wave * W + b: wave * W + b + 1, :], us.bitcast(F32))
                else:
                    nc.sync.dma_start(usb[b:b + 1, :], us)
            if kind == "final":
                return None
            uc = psT.tile([128, 2 * W], F32, tag="ucols", name=f"uc_{wave}_{si}")
            for m in range(2):
                nc.tensor.matmul(uc[:, m * W:(m + 1) * W],
                                 usb[:, m * 128:(m + 1) * 128], IWr)
            nxt = [tail_p.tile([128, W], F32R, tag=f"VC{m}", name=f"vc_{wave}_{si}_{m}")
                   for m in range(2)]
            for m in range(2):
                ucm = uc[:, m * W:(m + 1) * W]
                if kind == "comb":
                    ucs = tail_p.tile([128, W], F32, tag=f"ucs{m}", name=f"ucs_{wave}_{si}_{m}")
                    nc.scalar.copy(ucs, ucm)
                    nc.vector.scalar_tensor_tensor(nxt[m], cur[m], 3.0, ucs,
                                                   ALU.mult, ALU.subtract)
                elif kind == "a3":
                    # v4 = 3*v0 - 0.25*u
                    a3q = tail_p.tile([128, W], F32R, tag=f"a3q{m}", name=f"a3q_{wave}_{m}")
                    nc.scalar.mul(a3q, ucm, 0.25)
                    nc.vector.scalar_tensor_tensor(nxt[m], v0c[m], 3.0, a3q,
                                                   ALU.mult, ALU.subtract)
            return nxt

        for wave in range(N_WAVES):
            V0ROWS = rows_p.tile([W, D], F32R, tag="v0rows", name=f"v0rows_{wave}")
            mats = []
            for b in range(W):
                g = wave * W + b
                mats.append(phase_a(g, V0ROWS, b))

            # transpose v0 rows -> column tiles [128, W] x2
            v0ps = psT.tile([128, 2 * W], F32, tag="ucols", name=f"v0ps_{wave}")
            for m in range(2):
                nc.tensor.matmul(v0ps[:, m * W:(m + 1) * W],
                                 V0ROWS[:, m * 128:(m + 1) * 128], IWr)
            v0c = []
            for m in range(2):
                v = tail_p.tile([128, W], F32R, tag=f"VC{m}", name=f"v0c_{wave}_{m}")
                nc.scalar.copy(v, v0ps[:, m * W:(m + 1) * W])
                v0c.append(v)

            cur = v0c
            kinds = ["comb", "comb", "a3", "comb", "comb", "comb", "comb", "final"]
            for si in range(8):
                cur = matvec_step(si, wave, cur, mats, kinds[si],
                                  v0c=v0c if kinds[si] == "a3" else None)


_CACHED_NC = None


def _get_nc():
    global _CACHED_NC
    if _CACHED_NC is None:
        _CACHED_NC = build_module()
    return _CACHED_NC


def _run(feat, noise, **spmd_kwargs):
    feat = np.ascontiguousarray(np.asarray(feat), dtype=np.float32)
    noise = np.ascontiguousarray(np.asarray(noise), dtype=np.float32)
    cst, cstb, cstr = _const_arrays()
    nc = _get_nc()
    in_maps = []
    for c in range(N_CORES):
        in_maps.append({
            "feat": feat[c * ROWS_CORE:(c + 1) * ROWS_CORE],
            "noise": noise[c * ROWS_CORE:(c + 1) * ROWS_CORE],
            "cst": cst,
            "cstb": cstb,
            "cstr": cstr,
        })
    return run_bass_kernel_spmd(nc, in_maps, list(range(N_CORES)), **spmd_kwargs)


def kernel(feat, noise, n_per_graph):
    assert int(n_per_graph) == NPG
    try:
        res = _run(feat, noise)
    except Exception:
        # the axon device occasionally reports a transient unrecoverable
        # state; one retry usually succeeds
        res = _run(feat, noise)
    return np.concatenate([res.results[c]["out"] for c in range(N_CORES)], axis=0)



# revision 31
# speedup vs baseline: 1.1651x; 1.1651x over previous
"""DKEPooling Trainium2 kernel (v2).

Per-graph pipeline (d=256, n=512 nodes/graph):
  f = feat + 0.01*noise                    (cast+accum SWDGE DMA, bf16)
  C' = f^T f - s^T s / n                   (Gram + rank-1 in PSUM; s = colsum f)
  A  = C'/tr(C'); Newton-Schulz via the commuting-polynomial invariant
  T_{k+1} = 0.25 T_k (3I - T_k)^2          (6 matrix products per graph)
  followed by an 8-matvec tail applied to the mean.

Layout: every chain matrix is ONE [128, 512] tile (row-chunks side by side in
the free dim) = exactly one PSUM bank, so each stage is 4 matmuls + 1 evac +
1 DVE combine.  Tail matvec rows land at 32-aligned PSUM partitions (legal PE
column-group bases), transposed back to column form with a static selector
matmul; the per-graph output scale cb rides in the v0 selector values.

Sharding: data-parallel over graphs. 8 cores x 16 graphs; no cross-core comm.
"""
import numpy as np

import concourse.bacc as bacc
import concourse.bass as bass
import concourse.mybir as mybir
import concourse.tile as tile
from concourse.bass_utils import run_bass_kernel_spmd

F32 = mybir.dt.float32
BF16 = mybir.dt.bfloat16
F32R = mybir.dt.float32r
ALU = mybir.AluOpType
ACTF = mybir.ActivationFunctionType

N_CORES = 8
D = 256
NPG = 512
B_TOTAL = 128
B_CORE = B_TOTAL // N_CORES      # 16 graphs per core
ROWS_CORE = B_CORE * NPG         # 8192 feat rows per core
W = 4                            # graphs per tail wave
N_WAVES = B_CORE // W

# const tensor layout (f32 [128, 773]):
#   [:, 0:512]   = wide 3I: 3I block at cols 0:128 (chunk0) and 384:512 (chunk1)
#   [:, 512:640] = I128 (diag mask)
#   [:, 640:644] = M32: M32[32b, b] = 1  (row-selector for W=4 graphs)
#   [:, 644:772] = all-ones 128x128 block (fused trace reduce+broadcast)
#   [:, 772:788] = E4SEL: E4[c, 4b+j] = (c == b), c < 4 (tail row selector)
CST_COLS = 788


def _const_arrays():
    import ml_dtypes
    cst = np.zeros((128, CST_COLS), np.float32)
    eye = np.eye(128, dtype=np.float32)
    cst[:, 0:128] = 3.0 * eye
    cst[:, 384:512] = 3.0 * eye
    cst[:, 512:640] = eye
    for b in range(W):
        cst[32 * b, 640 + b] = 1.0
    cst[:, 644:772] = 1.0
    for b in range(W):
        cst[b, 772 + 4 * b: 772 + 4 * (b + 1)] = 1.0
    cstb = np.ones((128, 1), ml_dtypes.bfloat16)
    return cst, cstb


def _r(ap):
    return ap.bitcast(F32R)


def build_module():
    nc = bacc.Bacc(None, target_bir_lowering=False)
    feat_d = nc.declare_dram_parameter("feat", [ROWS_CORE, D], F32, isOutput=False)
    noise_d = nc.declare_dram_parameter("noise", [ROWS_CORE, D], F32, isOutput=False)
    cst_d = nc.declare_dram_parameter("cst", [128, CST_COLS], F32R, isOutput=False)
    cstb_d = nc.declare_dram_parameter("cstb", [128, 1], BF16, isOutput=False)
    out_d = nc.declare_dram_parameter("out", [B_CORE, D], F32, isOutput=True)

    with tile.TileContext(nc) as tc:
        _build_tile(tc, nc, feat_d, noise_d, cst_d, cstb_d, out_d)
    nc.compile()
    return nc


def _build_tile(tc, nc, feat_d, noise_d, cst_d, cstb_d, out_d):
    import contextlib
    import concourse.bass_isa as bass_isa
    ctx = contextlib.ExitStack()
    with ctx:
        g_p = ctx.enter_context(tc.tile_pool(name="gp", bufs=8))
        gc_p = ctx.enter_context(tc.tile_pool(name="gcp", bufs=4))
        mats_p = ctx.enter_context(tc.tile_pool(name="mats", bufs=9))
        chain_p = ctx.enter_context(tc.tile_pool(name="chain", bufs=4))
        small_p = ctx.enter_context(tc.tile_pool(name="small", bufs=12))
        tail_p = ctx.enter_context(tc.tile_pool(name="tailp", bufs=3))
        wave_p = ctx.enter_context(tc.tile_pool(name="wavep", bufs=2))
        cst_p = ctx.enter_context(tc.tile_pool(name="cstp", bufs=1))
        psGram = ctx.enter_context(tc.tile_pool(name="psGram", bufs=2, space="PSUM"))
        psStage = ctx.enter_context(tc.tile_pool(name="psStage", bufs=2, space="PSUM"))
        psS = ctx.enter_context(tc.tile_pool(name="psS", bufs=1, space="PSUM"))
        psRows = ctx.enter_context(tc.tile_pool(name="psRows", bufs=1, space="PSUM"))
        psTpc = ctx.enter_context(tc.tile_pool(name="psTpc", bufs=1, space="PSUM"))

        cst = cst_p.tile([128, CST_COLS], F32R, tag="cst", name="cst_sb")
        nc.gpsimd.dma_start(cst, cst_d[:, :])
        onesb = cst_p.tile([128, 1], BF16, tag="onesb", name="onesb_sb")
        nc.gpsimd.dma_start(onesb, cstb_d[:, :])

        cst3I = cst.bitcast(F32)[:, 0:512]
        I128 = cst.bitcast(F32)[:, 512:640]
        M32 = cst[:, 640:640 + W]          # f32r: matmul selector
        M32f = cst.bitcast(F32)[:, 640:640 + W]
        ones128 = cst.bitcast(F32)[:, 644:772]
        E4SEL = cst[:, 772:788]

        n_zeroed = {"s": 0}
        hook = globals().get("_DEBUG_HOOK", None) or (lambda name, ap: None)

        def mm256(dst_ps, L, R):
            """dst = L @ R for [128,512]-layout symmetric matrices (f32r APs)."""
            for m in range(2):
                for k in range(2):
                    nc.tensor.matmul(
                        dst_ps[:, m * D:(m + 1) * D],
                        L[:, k * D + m * 128: k * D + (m + 1) * 128],
                        R[:, k * D:(k + 1) * D],
                        start=(k == 0), stop=(k == 1))

        def phase_a1(g, s_ps, b):
            """Load graph g; column sums s into row 32b of the wave s bank."""
            gb = g_p.tile([128, 4 * D], BF16, tag="g", name=f"g_{g}")
            src = feat_d[g * NPG:(g + 1) * NPG, :].rearrange("(c p) d -> p c d", p=128)
            nsrc = noise_d[g * NPG:(g + 1) * NPG, :].rearrange("(c p) d -> p c d", p=128)
            nc.gpsimd.dma_start(gb, src)
            nc.gpsimd.dma_start(gb, nsrc, accum_op=ALU.add)
            for k in range(4):
                nc.tensor.matmul(s_ps[32 * b:32 * b + 1, 0:256], onesb,
                                 gb[:, k * D:(k + 1) * D],
                                 start=(k == 0), stop=(k == 3),
                                 tile_position=(0, 32 * b))
            return gb

        def phase_a2(g, gb, SB4, SBn4, b):
            """Gram + rank-1 into one [128,512] PSUM bank; per chunk m the
            accumulation group stays open from the first k-matmul until the
            rank-1 closes it (one open group per bank at a time)."""
            G = psGram.tile([128, 512], F32, tag="G", name=f"G_{g}")
            for m in range(2):
                for k in range(4):
                    nc.tensor.matmul(
                        G[:, m * D:(m + 1) * D],
                        gb[:, k * D + m * 128: k * D + (m + 1) * 128],
                        gb[:, k * D:(k + 1) * D],
                        start=(k == 0), stop=False)
                nc.tensor.matmul(G[:, m * D:(m + 1) * D],
                                 SBn4[32 * b:32 * b + 1, m * 128:(m + 1) * 128],
                                 SB4[32 * b:32 * b + 1, :],
                                 start=False, stop=True,
                                 tile_position=(32 * b, 0))
            return {"gb": gb, "G": G}

        def phase_b(wave, sts, s_ps, S4, SB4, SBn4):
            """rank-1 correction + trace + NS chain, stage-major across the
            wave's W graphs so independent graphs interleave on every engine."""
            gs = [wave * W + b for b in range(W)]
            Gcs, As, mats = [], [], [{} for _ in range(W)]

            # C' evacuation
            for b in range(W):
                G = sts[b]["G"]
                Gc = gc_p.tile([128, 512], F32R, tag="Gc", name=f"Gc_{gs[b]}")
                nc.scalar.copy(Gc, G)
                if gs[b] == 0:
                    hook("gc", Gc.bitcast(F32)[:, :])
                Gcs.append(Gc)

                        # trace: diag-mask partial sums (DVE) -> reduce + broadcast on PE
            # (via the widened s-bank: tr at [0, 256+b], bc at [:, 260+b])
            rcpbs, rcp2bs = [], []
            for b in range(W):
                g = gs[b]
                scr = small_p.tile([128, 128], F32, tag="scr", name=f"scr_{g}")
                dg = small_p.tile([128, 2], F32, tag="dg", name=f"dg_{g}")
                for m in range(2):
                    nc.vector.scalar_tensor_tensor(
                        scr, Gcs[b].bitcast(F32)[:, m * D + m * 128: m * D + (m + 1) * 128],
                        1.0, I128, ALU.mult, ALU.mult, accum_out=dg[:, m:m + 1])
                dgs = small_p.tile([128, 1], F32, tag="dgs", name=f"dgs_{g}")
                nc.vector.tensor_add(dgs, dg[:, 0:1], dg[:, 1:2])
                nc.tensor.matmul(s_ps[:, 260 + 4 * b:261 + 4 * b], ones128, dgs,
                                 start=True, stop=True)
                bc = s_ps[:, 260 + 4 * b:261 + 4 * b]
                rcpb = small_p.tile([128, 1], F32, tag="rcpb", name=f"rcpb_{g}")
                nc.vector.reciprocal(rcpb, bc)
                rcp2b = small_p.tile([128, 1], F32, tag="rcp2b", name=f"rcp2b_{g}")
                nc.vector.tensor_mul(rcp2b, rcpb, rcpb)
                # cb = sqrt(trc/(n-1)) * 0.03125/n, broadcast over partitions
                sq = small_p.tile([128, 1], F32, tag="sq", name=f"sq_{g}")
                nc.scalar.activation(sq, bc, ACTF.Sqrt, scale=1.0 / (NPG - 1))
                cb = small_p.tile([128, 1], F32, tag="cb", name=f"cb_{g}")
                nc.vector.tensor_scalar_mul(cb, sq, 0.03125 / NPG)
                rcpbs.append(rcpb)
                rcp2bs.append(rcp2b)
                mats[b]["cb"] = cb

            for b in range(W):
                A = mats_p.tile([128, 512], F32R, tag="A", name=f"A_{gs[b]}")
                nc.vector.tensor_scalar_mul(A, Gcs[b].bitcast(F32), rcpbs[b])
                if gs[b] == 0:
                    hook("a", A.bitcast(F32)[:, :])
                As.append(A)
                mats[b]["A"] = A

            # A2 (normalized via rcp^2 at evac)
            sta = [psStage.tile([128, 512], F32, tag="st", name=f"a2_{gs[b]}")
                   for b in range(W)]
            for b in range(W):
                mm256(sta[b], Gcs[b], Gcs[b])
            A2ns, W1s, V0s = [], [], []
            for b in range(W):
                A2n = chain_p.tile([128, 512], F32, tag="A2n", name=f"A2n_{gs[b]}")
                nc.scalar.activation(A2n, sta[b], ACTF.Copy, scale=rcp2bs[b])
                A2ns.append(A2n)
            for b in range(W):
                W1 = chain_p.tile([128, 512], F32R, tag="W1", name=f"W1_{gs[b]}")
                nc.vector.scalar_tensor_tensor(W1, As[b], 3.0, A2ns[b],
                                               ALU.mult, ALU.subtract)
                W1s.append(W1)
                V0 = chain_p.tile([128, 512], F32R, tag="V0", name=f"V0_{gs[b]}")
                nc.vector.scalar_tensor_tensor(V0, As[b], -1.0, cst3I,
                                               ALU.mult, ALU.add)
                V0s.append(V0)

            stp = [psStage.tile([128, 512], F32, tag="st", name=f"p_{gs[b]}")
                   for b in range(W)]
            T1s, V1s = [], []
            for b in range(W):
                mm256(stp[b], W1s[b], V0s[b])
            for b in range(W):
                T1 = mats_p.tile([128, 512], F32R, tag="T1", name=f"T1_{gs[b]}")
                nc.scalar.mul(T1, stp[b], 0.25)
                T1s.append(T1)
                mats[b]["T1"] = T1
            for b in range(W):
                V1 = chain_p.tile([128, 512], F32R, tag="V1", name=f"V1_{gs[b]}")
                nc.vector.scalar_tensor_tensor(V1, T1s[b], -1.0, cst3I,
                                               ALU.mult, ALU.add)
                V1s.append(V1)

            stq = [psStage.tile([128, 512], F32, tag="st", name=f"q_{gs[b]}")
                   for b in range(W)]
            Qbs = []
            for b in range(W):
                mm256(stq[b], T1s[b], V1s[b])
            for b in range(W):
                Qb = chain_p.tile([128, 512], F32R, tag="Qb", name=f"Qb_{gs[b]}")
                nc.scalar.copy(Qb, stq[b])
                Qbs.append(Qb)

            str_ = [psStage.tile([128, 512], F32, tag="st", name=f"rr_{gs[b]}")
                    for b in range(W)]
            T2s, V2s = [], []
            for b in range(W):
                mm256(str_[b], Qbs[b], V1s[b])
            for b in range(W):
                T2 = mats_p.tile([128, 512], F32R, tag="T2", name=f"T2_{gs[b]}")
                nc.scalar.mul(T2, str_[b], 0.25)
                T2s.append(T2)
                mats[b]["T2"] = T2
            for b in range(W):
                V2 = chain_p.tile([128, 512], F32R, tag="V2", name=f"V2_{gs[b]}")
                nc.vector.scalar_tensor_tensor(V2, T2s[b], -1.0, cst3I,
                                               ALU.mult, ALU.add)
                V2s.append(V2)

            sts5 = [psStage.tile([128, 512], F32, tag="st", name=f"s5_{gs[b]}")
                    for b in range(W)]
            Sbs = []
            for b in range(W):
                mm256(sts5[b], T2s[b], V2s[b])
            for b in range(W):
                Sb = chain_p.tile([128, 512], F32R, tag="Sb", name=f"Sb_{gs[b]}")
                nc.vector.tensor_copy(Sb, sts5[b])
                Sbs.append(Sb)

            stu = [psStage.tile([128, 512], F32, tag="st", name=f"u_{gs[b]}")
                   for b in range(W)]
            for b in range(W):
                mm256(stu[b], Sbs[b], V2s[b])
            for b in range(W):
                T3 = mats_p.tile([128, 512], F32R, tag="T3", name=f"T3_{gs[b]}")
                nc.scalar.mul(T3, stu[b], 0.25)
                if gs[b] == 0:
                    hook("t3", T3.bitcast(F32)[:, :])
                mats[b]["T3"] = T3
            return mats

        def tail_step(si, wave, cur, mats, kind, v0c3, tailidx):
            """One tail step for all W graphs.

            rows land at 32-aligned PSUM partitions, transposed back to
            column form [128, 2W] with the M32 selector."""
            xkey = {0: "T3", 1: "T3", 2: "T3", 3: "T3",
                    4: "T2", 5: "T1", 6: "A", 7: "A"}[si]
            # f32r matmuls require dst partition base 0: each graph's row
            # lands as a [W,256] block at partitions 0:W (junk rows = other
            # cur columns vs X_b), two graphs per PSUM bank.
            rows = [psRows.tile([W, 512], F32, tag=f"rows{h}",
                                name=f"rows{h}_{wave}_{si}") for h in range(2)]
            for b in range(W):
                X = mats[b][xkey]
                dst = rows[b // 2][:, (b % 2) * D:(b % 2 + 1) * D]
                for k in range(2):
                    nc.tensor.matmul(dst, cur[:, k * W:(k + 1) * W],
                                     X[:, k * D:(k + 1) * D],
                                     start=(k == 0), stop=(k == 1))
            if kind == "final":
                for h in range(2):
                    osb = tail_p.tile([W, 512], F32, tag=f"osb{h}",
                                      name=f"osb{h}_{wave}")
                    nc.scalar.copy(osb, rows[h])
                    for j in range(2):
                        b = 2 * h + j
                        nc.sync.dma_start(out_d[wave * W + b:wave * W + b + 1, :],
                                          osb[b:b + 1, j * D:(j + 1) * D])
                return None
            usb = []
            for h in range(2):
                u = tail_p.tile([W, 512], F32R, tag=f"usb{h}",
                                name=f"usb{h}_{wave}_{si}")
                if h == 0:
                    nc.scalar.copy(u, rows[h])
                else:
                    nc.vector.tensor_copy(u, rows[h])
                usb.append(u)
            tpc = psTpc.tile([128, 40], F32, tag="tpc", name=f"tpc_{wave}_{si}")
            for b in range(W):
                for m in range(2):
                    nc.tensor.matmul(tpc[:, (m * W + b) * 4:(m * W + b + 1) * 4],
                                     usb[b // 2][0:W, (b % 2) * D + m * 128:
                                                 (b % 2) * D + (m + 1) * 128],
                                     E4SEL[0:W, 4 * b:4 * (b + 1)],
                                     start=True, stop=True)
            ucs = tpc[:, 0:32].rearrange("p (c j) -> p c j", j=4)[:, :, 0]
            nxt = tail_p.tile([128, 2 * W], F32R, tag="cur",
                              name=f"cur_{wave}_{si}")
            if kind == "comb":
                nc.vector.scalar_tensor_tensor(nxt, cur, 3.0, ucs,
                                               ALU.mult, ALU.subtract)
            elif kind == "a3":
                nc.vector.scalar_tensor_tensor(nxt, ucs, -0.25, v0c3,
                                               ALU.mult, ALU.add)
            return nxt

        for wave in range(N_WAVES):
            s_ps = psS.tile([128, 276], F32, tag="s", name=f"s_{wave}")
            if n_zeroed["s"] < 1:
                n_zeroed["s"] += 1
                nc.scalar.memzero(s_ps)
            gbs = []
            for b in range(W):
                g = wave * W + b
                gbs.append(phase_a1(g, s_ps, b))
            # s evac + bf16 row tiles for the rank-1 update
            S4 = wave_p.tile([128, 256], F32R, tag="S4", name=f"S4_{wave}")
            nc.scalar.copy(S4, s_ps[:, 0:256])
            SB4 = wave_p.tile([128, 256], BF16, tag="SB4", name=f"SB4_{wave}")
            nc.gpsimd.tensor_copy(SB4, S4.bitcast(F32))
            SBn4 = wave_p.tile([128, 256], BF16, tag="SBn4", name=f"SBn4_{wave}")
            nc.gpsimd.tensor_scalar_mul(SBn4, SB4, -1.0 / NPG)
            sts = []
            for b in range(W):
                g = wave * W + b
                sts.append(phase_a2(g, gbs[b], SB4, SBn4, b))
            mats = phase_b(wave, sts, s_ps, S4, SB4, SBn4)

            # v0 columns via cb-valued selector
            E = wave_p.tile([128, W], F32R, tag="E", name=f"E_{wave}")
            for b in range(W):
                nc.vector.scalar_tensor_tensor(E[:, b:b + 1], mats[b]["cb"], 1.0,
                                               M32f[:, b:b + 1], ALU.mult, ALU.mult)
            tpv = psTpc.tile([128, 40], F32, tag="tpc", name=f"tpv_{wave}")
            for m in range(2):
                nc.tensor.matmul(tpv[:, 32 + m * W:32 + (m + 1) * W],
                                 S4[:, m * 128:(m + 1) * 128],
                                 E, start=True, stop=True)
            v0c = tail_p.tile([128, 2 * W], F32R, tag="cur", name=f"v0c_{wave}")
            nc.scalar.copy(v0c, tpv[:, 32:40])
            v0c3 = tail_p.tile([128, 2 * W], F32R, tag="v0c3", name=f"v0c3_{wave}")
            nc.vector.tensor_scalar_mul(v0c3, v0c, 3.0)

            if wave == 0:
                hook("s", s_ps[:, :])
                hook("v0", v0c.bitcast(F32)[:, :])
            cur = v0c
            kinds = ["comb", "comb", "a3", "comb", "comb", "comb", "comb", "final"]
            for si in range(8):
                cur = tail_step(si, wave, cur, mats, kinds[si], v0c3, si)
                if wave == 0 and si == 0:
                    hook("c1", cur.bitcast(F32)[:, :])


_CACHED_NC = None


def _get_nc():
    global _CACHED_NC
    if _CACHED_NC is None:
        _CACHED_NC = build_module()
    return _CACHED_NC


def _run(feat, noise, **spmd_kwargs):
    feat = np.ascontiguousarray(np.asarray(feat), dtype=np.float32)
    noise01 = np.asarray(noise, dtype=np.float32) * np.float32(0.01)
    noise01 = np.ascontiguousarray(noise01)
    cst, cstb = _const_arrays()
    nc = _get_nc()
    in_maps = []
    for c in range(N_CORES):
        in_maps.append({
            "feat": feat[c * ROWS_CORE:(c + 1) * ROWS_CORE],
            "noise": noise01[c * ROWS_CORE:(c + 1) * ROWS_CORE],
            "cst": cst,
            "cstb": cstb,
        })
    return run_bass_kernel_spmd(nc, in_maps, list(range(N_CORES)), **spmd_kwargs)


def kernel(feat, noise, n_per_graph):
    assert int(n_per_graph) == NPG
    try:
        res = _run(feat, noise)
    except Exception:
        # the axon device occasionally reports a transient unrecoverable
        # state; one retry usually succeeds
        res = _run(feat, noise)
    return np.concatenate([res.results[c]["out"] for c in range(N_CORES)], axis=0)


# revision 39
# speedup vs baseline: 1.4425x; 1.2381x over previous
"""DKEPooling Trainium2 kernel (v2).

Per-graph pipeline (d=256, n=512 nodes/graph):
  f = feat + 0.01*noise                    (cast+accum SWDGE DMA, bf16)
  C' = f^T f - s^T s / n                   (Gram + rank-1 in PSUM; s = colsum f)
  A  = C'/tr(C'); Newton-Schulz via the commuting-polynomial invariant
  T_{k+1} = 0.25 T_k (3I - T_k)^2          (6 matrix products per graph)
  followed by an 8-matvec tail applied to the mean.

Layout: every chain matrix is ONE [128, 512] tile (row-chunks side by side in
the free dim) = exactly one PSUM bank, so each stage is 4 matmuls + 1 evac +
1 DVE combine.  Tail matvec rows land at 32-aligned PSUM partitions (legal PE
column-group bases), transposed back to column form with a static selector
matmul; the per-graph output scale cb rides in the v0 selector values.

Sharding: data-parallel over graphs. 8 cores x 16 graphs; no cross-core comm.
"""
import numpy as np

import concourse.bacc as bacc
import concourse.bass as bass
import concourse.mybir as mybir
import concourse.tile as tile
from concourse.bass_utils import run_bass_kernel_spmd

F32 = mybir.dt.float32
BF16 = mybir.dt.bfloat16
F32R = mybir.dt.float32r
ALU = mybir.AluOpType
ACTF = mybir.ActivationFunctionType

N_CORES = 8
D = 256
NPG = 512
B_TOTAL = 128
B_CORE = B_TOTAL // N_CORES      # 16 graphs per core
ROWS_CORE = B_CORE * NPG         # 8192 feat rows per core
W = 4                            # graphs per tail wave
N_WAVES = B_CORE // W

# const tensor layout (f32 [128, 773]):
#   [:, 0:512]   = wide 3I: 3I block at cols 0:128 (chunk0) and 384:512 (chunk1)
#   [:, 512:640] = I128 (diag mask)
#   [:, 640:644] = M32: M32[32b, b] = 1  (row-selector for W=4 graphs)
#   [:, 644:772] = all-ones 128x128 block (fused trace reduce+broadcast)
#   [:, 772:788] = E4SEL: E4[c, 4b+j] = (c == b), c < 4 (tail row selector)
CST_COLS = 788


def _const_arrays():
    import ml_dtypes
    cst = np.zeros((128, CST_COLS), np.float32)
    eye = np.eye(128, dtype=np.float32)
    cst[:, 0:128] = 3.0 * eye
    cst[:, 384:512] = 3.0 * eye
    cst[:, 512:640] = eye
    for b in range(W):
        cst[32 * b, 640 + b] = 1.0
    cst[:, 644:772] = 1.0
    for b in range(W):
        cst[b, 772 + 4 * b: 772 + 4 * (b + 1)] = 1.0
    cstb = np.ones((128, 1), ml_dtypes.bfloat16)
    return cst, cstb


def _r(ap):
    return ap.bitcast(F32R)


def build_module():
    nc = bacc.Bacc(None, target_bir_lowering=False)
    feat_d = nc.declare_dram_parameter("feat", [ROWS_CORE, D], F32, isOutput=False)
    noise_d = nc.declare_dram_parameter("noise", [ROWS_CORE, D], F32, isOutput=False)
    cst_d = nc.declare_dram_parameter("cst", [128, CST_COLS], F32R, isOutput=False)
    cstb_d = nc.declare_dram_parameter("cstb", [128, 1], BF16, isOutput=False)
    out_d = nc.declare_dram_parameter("out", [B_CORE, D], F32, isOutput=True)

    with tile.TileContext(nc) as tc:
        _build_tile(tc, nc, feat_d, noise_d, cst_d, cstb_d, out_d)
    nc.compile()
    return nc


def _build_tile(tc, nc, feat_d, noise_d, cst_d, cstb_d, out_d):
    import contextlib
    import concourse.bass_isa as bass_isa
    ctx = contextlib.ExitStack()
    with ctx:
        g_p = ctx.enter_context(tc.tile_pool(name="gp", bufs=8))
        st0_p = ctx.enter_context(tc.tile_pool(name="st0p", bufs=2))
        gc_p = ctx.enter_context(tc.tile_pool(name="gcp", bufs=4))
        mats_p = ctx.enter_context(tc.tile_pool(name="mats", bufs=9))
        chain_p = ctx.enter_context(tc.tile_pool(name="chain", bufs=4))
        small_p = ctx.enter_context(tc.tile_pool(name="small", bufs=12))
        tail_p = ctx.enter_context(tc.tile_pool(name="tailp", bufs=3))
        wave_p = ctx.enter_context(tc.tile_pool(name="wavep", bufs=2))
        cst_p = ctx.enter_context(tc.tile_pool(name="cstp", bufs=1))
        psGram = ctx.enter_context(tc.tile_pool(name="psGram", bufs=2, space="PSUM"))
        psStage = ctx.enter_context(tc.tile_pool(name="psStage", bufs=2, space="PSUM"))
        psS = ctx.enter_context(tc.tile_pool(name="psS", bufs=1, space="PSUM"))
        psRows = ctx.enter_context(tc.tile_pool(name="psRows", bufs=1, space="PSUM"))
        psTpc = ctx.enter_context(tc.tile_pool(name="psTpc", bufs=1, space="PSUM"))

        onesb = cst_p.tile([128, 1], BF16, tag="onesb", name="onesb_sb")
        nc.gpsimd.dma_start(onesb, cstb_d[:, :])
        cst = cst_p.tile([128, CST_COLS], F32R, tag="cst", name="cst_sb")
        cst_loaded = [False]

        def load_cst():
            if not cst_loaded[0]:
                cst_loaded[0] = True
                nc.scalar.dma_start(cst, cst_d[:, :])

        cst3I = cst.bitcast(F32)[:, 0:512]
        I128 = cst.bitcast(F32)[:, 512:640]
        M32 = cst[:, 640:640 + W]          # f32r: matmul selector
        M32f = cst.bitcast(F32)[:, 640:640 + W]
        ones128 = cst.bitcast(F32)[:, 644:772]
        E4SEL = cst[:, 772:788]

        n_zeroed = {"s": 0}
        hook = globals().get("_DEBUG_HOOK", None) or (lambda name, ap: None)

        def mm256(dst_ps, L, R):
            """dst = L @ R for [128,512]-layout symmetric matrices (f32r APs)."""
            for m in range(2):
                for k in range(2):
                    nc.tensor.matmul(
                        dst_ps[:, m * D:(m + 1) * D],
                        L[:, k * D + m * 128: k * D + (m + 1) * 128],
                        R[:, k * D:(k + 1) * D],
                        start=(k == 0), stop=(k == 1))

        def phase_a1(g, s_ps, b):
            """Load graph g; column sums s into row 32b of the wave s bank."""
            gb = g_p.tile([128, 4 * D], BF16, tag="g", name=f"g_{g}")
            src = feat_d[g * NPG:(g + 1) * NPG, :].rearrange("(c p) d -> p c d", p=128)
            nsrc = noise_d[g * NPG:(g + 1) * NPG, :].rearrange("(c p) d -> p c d", p=128)
            if g < 2:
                # wave 0: HWDGE per-chunk f32 loads (SP + ACT issue in
                # parallel, no Pool descriptor serialization) + DVE combine;
                # SWDGE cast+accum loads would gate the first Gram by ~7us.
                ft = st0_p.tile([128, 4 * D], F32, tag="ft0", name=f"ft0_{g}")
                nz = st0_p.tile([128, 4 * D], F32, tag="nz0", name=f"nz0_{g}")
                for c in range(4):
                    nc.sync.dma_start(ft[:, c * D:(c + 1) * D], src[:, c, :])
                    nc.scalar.dma_start(nz[:, c * D:(c + 1) * D], nsrc[:, c, :])
                for c in range(4):
                    nc.vector.tensor_add(gb[:, c * D:(c + 1) * D],
                                         ft[:, c * D:(c + 1) * D],
                                         nz[:, c * D:(c + 1) * D])
            else:
                nc.gpsimd.dma_start(gb, src)
                nc.gpsimd.dma_start(gb, nsrc, accum_op=ALU.add)
            for k in range(4):
                nc.tensor.matmul(s_ps[32 * b:32 * b + 1, 0:256], onesb,
                                 gb[:, k * D:(k + 1) * D],
                                 start=(k == 0), stop=(k == 3),
                                 tile_position=(0, 32 * b))
            return gb

        def phase_a2(g, gb, SB4, SBn4, b):
            """Gram + rank-1 into one [128,512] PSUM bank; per chunk m the
            accumulation group stays open from the first k-matmul until the
            rank-1 closes it (one open group per bank at a time)."""
            G = psGram.tile([128, 512], F32, tag="G", name=f"G_{g}")
            for m in range(2):
                for k in range(4):
                    nc.tensor.matmul(
                        G[:, m * D:(m + 1) * D],
                        gb[:, k * D + m * 128: k * D + (m + 1) * 128],
                        gb[:, k * D:(k + 1) * D],
                        start=(k == 0), stop=False)
                nc.tensor.matmul(G[:, m * D:(m + 1) * D],
                                 SBn4[32 * b:32 * b + 1, m * 128:(m + 1) * 128],
                                 SB4[32 * b:32 * b + 1, :],
                                 start=False, stop=True,
                                 tile_position=(32 * b, 0))
            return {"gb": gb, "G": G}

        def phase_b(wave, sts, s_ps, S4, SB4, SBn4):
            """rank-1 correction + trace + NS chain, stage-major across the
            wave's W graphs so independent graphs interleave on every engine."""
            gs = [wave * W + b for b in range(W)]
            Gcs, As, mats = [], [], [{} for _ in range(W)]

            # C' evacuation
            for b in range(W):
                G = sts[b]["G"]
                Gc = gc_p.tile([128, 512], F32R, tag="Gc", name=f"Gc_{gs[b]}")
                nc.scalar.copy(Gc, G)
                if gs[b] == 0:
                    hook("gc", Gc.bitcast(F32)[:, :])
                Gcs.append(Gc)

                        # trace: diag-mask partial sums (DVE) -> reduce + broadcast on PE
            # (via the widened s-bank: tr at [0, 256+b], bc at [:, 260+b])
            rcpbs, rcp2bs = [], []
            for b in range(W):
                g = gs[b]
                scr = small_p.tile([128, 128], F32, tag="scr", name=f"scr_{g}")
                dg = small_p.tile([128, 2], F32, tag="dg", name=f"dg_{g}")
                for m in range(2):
                    nc.vector.scalar_tensor_tensor(
                        scr, Gcs[b].bitcast(F32)[:, m * D + m * 128: m * D + (m + 1) * 128],
                        1.0, I128, ALU.mult, ALU.mult, accum_out=dg[:, m:m + 1])
                dgs = small_p.tile([128, 1], F32, tag="dgs", name=f"dgs_{g}")
                nc.vector.tensor_add(dgs, dg[:, 0:1], dg[:, 1:2])
                nc.tensor.matmul(s_ps[:, 260 + 4 * b:261 + 4 * b], ones128, dgs,
                                 start=True, stop=True)
                bc = s_ps[:, 260 + 4 * b:261 + 4 * b]
                rcpb = small_p.tile([128, 1], F32, tag="rcpb", name=f"rcpb_{g}")
                nc.vector.reciprocal(rcpb, bc)
                rcp2b = small_p.tile([128, 1], F32, tag="rcp2b", name=f"rcp2b_{g}")
                nc.vector.tensor_mul(rcp2b, rcpb, rcpb)
                # cb = sqrt(trc/(n-1)) * 0.03125/n, broadcast over partitions
                sq = small_p.tile([128, 1], F32, tag="sq", name=f"sq_{g}")
                nc.scalar.activation(sq, bc, ACTF.Sqrt, scale=1.0 / (NPG - 1))
                cb = small_p.tile([128, 1], F32, tag="cb", name=f"cb_{g}")
                nc.vector.tensor_scalar_mul(cb, sq, 0.03125 / NPG)
                rcpbs.append(rcpb)
                rcp2bs.append(rcp2b)
                mats[b]["cb"] = cb

            for b in range(W):
                A = mats_p.tile([128, 512], F32R, tag="A", name=f"A_{gs[b]}")
                nc.vector.tensor_scalar_mul(A, Gcs[b].bitcast(F32), rcpbs[b])
                if gs[b] == 0:
                    hook("a", A.bitcast(F32)[:, :])
                As.append(A)
                mats[b]["A"] = A

            # A2 (normalized via rcp^2 at evac)
            sta = [psStage.tile([128, 512], F32, tag="st", name=f"a2_{gs[b]}")
                   for b in range(W)]
            for b in range(W):
                mm256(sta[b], Gcs[b], Gcs[b])
            A2ns, W1s, V0s = [], [], []
            for b in range(W):
                A2n = chain_p.tile([128, 512], F32, tag="A2n", name=f"A2n_{gs[b]}")
                nc.scalar.activation(A2n, sta[b], ACTF.Copy, scale=rcp2bs[b])
                A2ns.append(A2n)
            for b in range(W):
                W1 = chain_p.tile([128, 512], F32R, tag="W1", name=f"W1_{gs[b]}")
                nc.vector.scalar_tensor_tensor(W1, As[b], 3.0, A2ns[b],
                                               ALU.mult, ALU.subtract)
                W1s.append(W1)
                V0 = chain_p.tile([128, 512], F32R, tag="V0", name=f"V0_{gs[b]}")
                nc.vector.scalar_tensor_tensor(V0, As[b], -1.0, cst3I,
                                               ALU.mult, ALU.add)
                V0s.append(V0)

            stp = [psStage.tile([128, 512], F32, tag="st", name=f"p_{gs[b]}")
                   for b in range(W)]
            T1s, V1s = [], []
            for b in range(W):
                mm256(stp[b], W1s[b], V0s[b])
            for b in range(W):
                T1 = mats_p.tile([128, 512], F32R, tag="T1", name=f"T1_{gs[b]}")
                nc.scalar.mul(T1, stp[b], 0.25)
                T1s.append(T1)
                mats[b]["T1"] = T1
            for b in range(W):
                V1 = chain_p.tile([128, 512], F32R, tag="V1", name=f"V1_{gs[b]}")
                nc.vector.scalar_tensor_tensor(V1, T1s[b], -1.0, cst3I,
                                               ALU.mult, ALU.add)
                V1s.append(V1)

            stq = [psStage.tile([128, 512], F32, tag="st", name=f"q_{gs[b]}")
                   for b in range(W)]
            Qbs = []
            for b in range(W):
                mm256(stq[b], T1s[b], V1s[b])
            for b in range(W):
                Qb = chain_p.tile([128, 512], F32R, tag="Qb", name=f"Qb_{gs[b]}")
                nc.scalar.copy(Qb, stq[b])
                Qbs.append(Qb)

            str_ = [psStage.tile([128, 512], F32, tag="st", name=f"rr_{gs[b]}")
                    for b in range(W)]
            T2s, V2s = [], []
            for b in range(W):
                mm256(str_[b], Qbs[b], V1s[b])
            for b in range(W):
                T2 = mats_p.tile([128, 512], F32R, tag="T2", name=f"T2_{gs[b]}")
                nc.scalar.mul(T2, str_[b], 0.25)
                T2s.append(T2)
                mats[b]["T2"] = T2
            for b in range(W):
                V2 = chain_p.tile([128, 512], F32R, tag="V2", name=f"V2_{gs[b]}")
                nc.vector.scalar_tensor_tensor(V2, T2s[b], -1.0, cst3I,
                                               ALU.mult, ALU.add)
                V2s.append(V2)

            sts5 = [psStage.tile([128, 512], F32, tag="st", name=f"s5_{gs[b]}")
                    for b in range(W)]
            Sbs = []
            for b in range(W):
                mm256(sts5[b], T2s[b], V2s[b])
            for b in range(W):
                Sb = chain_p.tile([128, 512], F32R, tag="Sb", name=f"Sb_{gs[b]}")
                nc.vector.tensor_copy(Sb, sts5[b])
                Sbs.append(Sb)

            stu = [psStage.tile([128, 512], F32, tag="st", name=f"u_{gs[b]}")
                   for b in range(W)]
            for b in range(W):
                mm256(stu[b], Sbs[b], V2s[b])
            for b in range(W):
                T3 = mats_p.tile([128, 512], F32R, tag="T3", name=f"T3_{gs[b]}")
                nc.scalar.mul(T3, stu[b], 0.25)
                if gs[b] == 0:
                    hook("t3", T3.bitcast(F32)[:, :])
                mats[b]["T3"] = T3
            return mats

        def tail_step(si, wave, cur, mats, kind, v0c3, tailidx):
            """One tail step for all W graphs.

            rows land at 32-aligned PSUM partitions, transposed back to
            column form [128, 2W] with the M32 selector."""
            xkey = {0: "T3", 1: "T3", 2: "T3", 3: "T3",
                    4: "T2", 5: "T1", 6: "A", 7: "A"}[si]
            # f32r matmuls require dst partition base 0: each graph's row
            # lands as a [W,256] block at partitions 0:W (junk rows = other
            # cur columns vs X_b), two graphs per PSUM bank.
            rows = [psRows.tile([W, 512], F32, tag=f"rows{h}",
                                name=f"rows{h}_{wave}_{si}") for h in range(2)]
            for b in range(W):
                X = mats[b][xkey]
                dst = rows[b // 2][:, (b % 2) * D:(b % 2 + 1) * D]
                for k in range(2):
                    nc.tensor.matmul(dst, cur[:, k * W:(k + 1) * W],
                                     X[:, k * D:(k + 1) * D],
                                     start=(k == 0), stop=(k == 1))
            if kind == "final":
                for h in range(2):
                    osb = tail_p.tile([W, 512], F32, tag=f"osb{h}",
                                      name=f"osb{h}_{wave}")
                    nc.scalar.copy(osb, rows[h])
                    for j in range(2):
                        b = 2 * h + j
                        nc.sync.dma_start(out_d[wave * W + b:wave * W + b + 1, :],
                                          osb[b:b + 1, j * D:(j + 1) * D])
                return None
            usb = []
            for h in range(2):
                u = tail_p.tile([W, 512], F32R, tag=f"usb{h}",
                                name=f"usb{h}_{wave}_{si}")
                if h == 0:
                    nc.scalar.copy(u, rows[h])
                else:
                    nc.vector.tensor_copy(u, rows[h])
                usb.append(u)
            tpc = psTpc.tile([128, 40], F32, tag="tpc", name=f"tpc_{wave}_{si}")
            for b in range(W):
                for m in range(2):
                    nc.tensor.matmul(tpc[:, (m * W + b) * 4:(m * W + b + 1) * 4],
                                     usb[b // 2][0:W, (b % 2) * D + m * 128:
                                                 (b % 2) * D + (m + 1) * 128],
                                     E4SEL[0:W, 4 * b:4 * (b + 1)],
                                     start=True, stop=True)
            ucs = tpc[:, 0:32].rearrange("p (c j) -> p c j", j=4)[:, :, 0]
            nxt = tail_p.tile([128, 2 * W], F32R, tag="cur",
                              name=f"cur_{wave}_{si}")
            if kind == "comb":
                nc.vector.scalar_tensor_tensor(nxt, cur, 3.0, ucs,
                                               ALU.mult, ALU.subtract)
            elif kind == "a3":
                nc.vector.scalar_tensor_tensor(nxt, ucs, -0.25, v0c3,
                                               ALU.mult, ALU.add)
            return nxt

        for wave in range(N_WAVES):
            s_ps = psS.tile([128, 276], F32, tag="s", name=f"s_{wave}")
            if n_zeroed["s"] < 1:
                n_zeroed["s"] += 1
                nc.scalar.memzero(s_ps)
            gbs = []
            for b in range(W):
                g = wave * W + b
                gbs.append(phase_a1(g, s_ps, b))
            load_cst()
            # s evac + bf16 row tiles for the rank-1 update
            S4 = wave_p.tile([128, 256], F32R, tag="S4", name=f"S4_{wave}")
            nc.scalar.copy(S4, s_ps[:, 0:256])
            SB4 = wave_p.tile([128, 256], BF16, tag="SB4", name=f"SB4_{wave}")
            nc.scalar.copy(SB4, s_ps[:, 0:256])
            SBn4 = wave_p.tile([128, 256], BF16, tag="SBn4", name=f"SBn4_{wave}")
            nc.vector.tensor_scalar_mul(SBn4, SB4, -1.0 / NPG)
            sts = []
            for b in range(W):
                g = wave * W + b
                sts.append(phase_a2(g, gbs[b], SB4, SBn4, b))
            mats = phase_b(wave, sts, s_ps, S4, SB4, SBn4)

            # v0 columns via cb-valued selector
            E = wave_p.tile([128, W], F32R, tag="E", name=f"E_{wave}")
            for b in range(W):
                nc.vector.scalar_tensor_tensor(E[:, b:b + 1], mats[b]["cb"], 1.0,
                                               M32f[:, b:b + 1], ALU.mult, ALU.mult)
            tpv = psTpc.tile([128, 40], F32, tag="tpc", name=f"tpv_{wave}")
            for m in range(2):
                nc.tensor.matmul(tpv[:, 32 + m * W:32 + (m + 1) * W],
                                 S4[:, m * 128:(m + 1) * 128],
                                 E, start=True, stop=True)
            v0c = tail_p.tile([128, 2 * W], F32R, tag="cur", name=f"v0c_{wave}")
            nc.scalar.copy(v0c, tpv[:, 32:40])
            v0c3 = tail_p.tile([128, 2 * W], F32R, tag="v0c3", name=f"v0c3_{wave}")
            nc.vector.tensor_scalar_mul(v0c3, v0c, 3.0)

            if wave == 0:
                hook("s", s_ps[:, :])
                hook("v0", v0c.bitcast(F32)[:, :])
            cur = v0c
            kinds = ["comb", "comb", "a3", "comb", "comb", "comb", "comb", "final"]
            for si in range(8):
                cur = tail_step(si, wave, cur, mats, kinds[si], v0c3, si)
                if wave == 0 and si == 0:
                    hook("c1", cur.bitcast(F32)[:, :])


_CACHED_NC = None


def _get_nc():
    global _CACHED_NC
    if _CACHED_NC is None:
        _CACHED_NC = build_module()
    return _CACHED_NC


def _run(feat, noise, **spmd_kwargs):
    feat = np.ascontiguousarray(np.asarray(feat), dtype=np.float32)
    noise01 = np.asarray(noise, dtype=np.float32) * np.float32(0.01)
    noise01 = np.ascontiguousarray(noise01)
    cst, cstb = _const_arrays()
    nc = _get_nc()
    in_maps = []
    for c in range(N_CORES):
        in_maps.append({
            "feat": feat[c * ROWS_CORE:(c + 1) * ROWS_CORE],
            "noise": noise01[c * ROWS_CORE:(c + 1) * ROWS_CORE],
            "cst": cst,
            "cstb": cstb,
        })
    return run_bass_kernel_spmd(nc, in_maps, list(range(N_CORES)), **spmd_kwargs)


def kernel(feat, noise, n_per_graph):
    assert int(n_per_graph) == NPG
    try:
        res = _run(feat, noise)
    except Exception:
        # the axon device occasionally reports a transient unrecoverable
        # state; one retry usually succeeds
        res = _run(feat, noise)
    return np.concatenate([res.results[c]["out"] for c in range(N_CORES)], axis=0)


# revision 44
# speedup vs baseline: 1.6276x; 1.1283x over previous
"""DKEPooling Trainium2 kernel (v2).

Per-graph pipeline (d=256, n=512 nodes/graph):
  f = feat + 0.01*noise     (one SWDGE cast DMA + one cast+accum DMA -> bf16;
                             the host pre-scales noise by 0.01)
  C' = f^T f - s^T s / n    (Gram + rank-1 in PSUM; s = colsum f via ones-matmul)
  A  = C'/tr(C')            (trace = diag-mask partials + one fused
                             reduce-and-broadcast matmul with an all-ones block)
  Newton-Schulz via the commuting-polynomial invariant
  T_{k+1} = 0.25 T_k (3I - T_k)^2   (6 matrix products per graph), then an
  8-matvec tail applied to the mean.

Layout: every chain matrix is ONE [128, 512] tile (row-chunks side by side in
the free dim) = exactly one PSUM bank, so each stage is 4 matmuls + 1 evac +
1 DVE combine.  PSUM rule learned on HW: only one OPEN accumulation group per
bank at a time (interleaved groups corrupt each other), so the Gram keeps each
chunk's group open until the rank-1 update closes it.  f32r matmuls also
require dst partition base 0: tail matvec rows land as [W,256] blocks at
partitions 0:W (junk rows are the other graphs' columns), two graphs per bank,
and tiny [4,4] selector matmuls transpose the right row back to column form;
the per-graph output scale cb rides in the v0 selector values.

Sharding: data-parallel over graphs. 8 cores x 16 graphs; no cross-core comm.
"""
import numpy as np

import concourse.bacc as bacc
import concourse.bass as bass
import concourse.mybir as mybir
import concourse.tile as tile
from concourse.bass_utils import run_bass_kernel_spmd

F32 = mybir.dt.float32
BF16 = mybir.dt.bfloat16
F32R = mybir.dt.float32r
ALU = mybir.AluOpType
ACTF = mybir.ActivationFunctionType

N_CORES = 8
D = 256
NPG = 512
B_TOTAL = 128
B_CORE = B_TOTAL // N_CORES      # 16 graphs per core
ROWS_CORE = B_CORE * NPG         # 8192 feat rows per core
W = 4                            # graphs per tail wave
N_WAVES = B_CORE // W

# const tensor layout (f32 [128, 773]):
#   [:, 0:512]   = wide 3I: 3I block at cols 0:128 (chunk0) and 384:512 (chunk1)
#   [:, 512:640] = I128 (diag mask)
#   [:, 640:644] = M32: M32[32b, b] = 1  (row-selector for W=4 graphs)
#   [:, 644:772] = all-ones 128x128 block (fused trace reduce+broadcast)
#   [:, 772:788] = E4SEL: E4[c, 4b+j] = (c == b), c < 4 (tail row selector)
CST_COLS = 788


def _const_arrays():
    import ml_dtypes
    cst = np.zeros((128, CST_COLS), np.float32)
    eye = np.eye(128, dtype=np.float32)
    cst[:, 0:128] = 3.0 * eye
    cst[:, 384:512] = 3.0 * eye
    cst[:, 512:640] = eye
    for b in range(W):
        cst[32 * b, 640 + b] = 1.0
    cst[:, 644:772] = 1.0
    for b in range(W):
        cst[b, 772 + 4 * b: 772 + 4 * (b + 1)] = 1.0
    cstb = np.ones((128, 1), ml_dtypes.bfloat16)
    return cst, cstb


def _r(ap):
    return ap.bitcast(F32R)


def build_module():
    nc = bacc.Bacc(None, target_bir_lowering=False)
    feat_d = nc.declare_dram_parameter("feat", [ROWS_CORE, D], F32, isOutput=False)
    noise_d = nc.declare_dram_parameter("noise", [ROWS_CORE, D], F32, isOutput=False)
    cst_d = nc.declare_dram_parameter("cst", [128, CST_COLS], F32R, isOutput=False)
    cstb_d = nc.declare_dram_parameter("cstb", [128, 1], BF16, isOutput=False)
    out_d = nc.declare_dram_parameter("out", [B_CORE, D], F32, isOutput=True)

    with tile.TileContext(nc) as tc:
        _build_tile(tc, nc, feat_d, noise_d, cst_d, cstb_d, out_d)
    nc.compile()
    return nc


def _build_tile(tc, nc, feat_d, noise_d, cst_d, cstb_d, out_d):
    import contextlib
    import concourse.bass_isa as bass_isa
    ctx = contextlib.ExitStack()
    with ctx:
        g_p = ctx.enter_context(tc.tile_pool(name="gp", bufs=8))
        st0_p = ctx.enter_context(tc.tile_pool(name="st0p", bufs=2))
        gc_p = ctx.enter_context(tc.tile_pool(name="gcp", bufs=4))
        mats_p = ctx.enter_context(tc.tile_pool(name="mats", bufs=9))
        chain_p = ctx.enter_context(tc.tile_pool(name="chain", bufs=4))
        small_p = ctx.enter_context(tc.tile_pool(name="small", bufs=12))
        tail_p = ctx.enter_context(tc.tile_pool(name="tailp", bufs=3))
        wave_p = ctx.enter_context(tc.tile_pool(name="wavep", bufs=2))
        cst_p = ctx.enter_context(tc.tile_pool(name="cstp", bufs=1))
        psGram = ctx.enter_context(tc.tile_pool(name="psGram", bufs=2, space="PSUM"))
        psStage = ctx.enter_context(tc.tile_pool(name="psStage", bufs=2, space="PSUM"))
        psS = ctx.enter_context(tc.tile_pool(name="psS", bufs=1, space="PSUM"))
        psRows = ctx.enter_context(tc.tile_pool(name="psRows", bufs=1, space="PSUM"))
        psTpc = ctx.enter_context(tc.tile_pool(name="psTpc", bufs=1, space="PSUM"))

        onesb = cst_p.tile([128, 1], BF16, tag="onesb", name="onesb_sb")
        nc.gpsimd.dma_start(onesb, cstb_d[:, :])
        cst = cst_p.tile([128, CST_COLS], F32R, tag="cst", name="cst_sb")
        cst_loaded = [False]

        def load_cst():
            if not cst_loaded[0]:
                cst_loaded[0] = True
                nc.scalar.dma_start(cst, cst_d[:, :])

        cst3I = cst.bitcast(F32)[:, 0:512]
        I128 = cst.bitcast(F32)[:, 512:640]
        M32 = cst[:, 640:640 + W]          # f32r: matmul selector
        M32f = cst.bitcast(F32)[:, 640:640 + W]
        ones128 = cst.bitcast(F32)[:, 644:772]
        E4SEL = cst[:, 772:788]

        n_zeroed = {"s": 0}
        hook = globals().get("_DEBUG_HOOK", None) or (lambda name, ap: None)

        def mm256(dst_ps, L, R):
            """dst = L @ R for [128,512]-layout symmetric matrices (f32r APs)."""
            for m in range(2):
                for k in range(2):
                    nc.tensor.matmul(
                        dst_ps[:, m * D:(m + 1) * D],
                        L[:, k * D + m * 128: k * D + (m + 1) * 128],
                        R[:, k * D:(k + 1) * D],
                        start=(k == 0), stop=(k == 1))

        def phase_a1(g, s_ps, b):
            """Load graph g; column sums s into row 32b of the wave s bank."""
            gb = g_p.tile([128, 4 * D], BF16, tag="g", name=f"g_{g}")
            src = feat_d[g * NPG:(g + 1) * NPG, :].rearrange("(c p) d -> p c d", p=128)
            nsrc = noise_d[g * NPG:(g + 1) * NPG, :].rearrange("(c p) d -> p c d", p=128)
            if g < 2:
                # wave 0: HWDGE per-chunk f32 loads (SP + ACT issue in
                # parallel, no Pool descriptor serialization) + DVE combine;
                # SWDGE cast+accum loads would gate the first Gram by ~7us.
                ft = st0_p.tile([128, 4 * D], F32, tag="ft0", name=f"ft0_{g}")
                nz = st0_p.tile([128, 4 * D], F32, tag="nz0", name=f"nz0_{g}")
                for c in range(4):
                    nc.sync.dma_start(ft[:, c * D:(c + 1) * D], src[:, c, :])
                    nc.scalar.dma_start(nz[:, c * D:(c + 1) * D], nsrc[:, c, :])
                for c in range(4):
                    nc.vector.tensor_add(gb[:, c * D:(c + 1) * D],
                                         ft[:, c * D:(c + 1) * D],
                                         nz[:, c * D:(c + 1) * D])
            else:
                nc.gpsimd.dma_start(gb, src)
                nc.gpsimd.dma_start(gb, nsrc, accum_op=ALU.add)
            for k in range(4):
                nc.tensor.matmul(s_ps[32 * b:32 * b + 1, 0:256], onesb,
                                 gb[:, k * D:(k + 1) * D],
                                 start=(k == 0), stop=(k == 3),
                                 tile_position=(0, 32 * b))
            return gb

        def phase_a2(g, gb, SB4, SBn4, b):
            """Gram + rank-1 into one [128,512] PSUM bank; per chunk m the
            accumulation group stays open from the first k-matmul until the
            rank-1 closes it (one open group per bank at a time)."""
            G = psGram.tile([128, 512], F32, tag="G", name=f"G_{g}")
            for m in range(2):
                for k in range(4):
                    nc.tensor.matmul(
                        G[:, m * D:(m + 1) * D],
                        gb[:, k * D + m * 128: k * D + (m + 1) * 128],
                        gb[:, k * D:(k + 1) * D],
                        start=(k == 0), stop=False)
                nc.tensor.matmul(G[:, m * D:(m + 1) * D],
                                 SBn4[32 * b:32 * b + 1, m * 128:(m + 1) * 128],
                                 SB4[32 * b:32 * b + 1, :],
                                 start=False, stop=True,
                                 tile_position=(32 * b, 0))
            return {"gb": gb, "G": G}

        def phase_b(wave, sts, s_ps, S4, SB4, SBn4):
            """rank-1 correction + trace + NS chain, stage-major across the
            wave's W graphs so independent graphs interleave on every engine."""
            gs = [wave * W + b for b in range(W)]
            Gcs, As, mats = [], [], [{} for _ in range(W)]

            # C' evacuation
            for b in range(W):
                G = sts[b]["G"]
                Gc = gc_p.tile([128, 512], F32R, tag="Gc", name=f"Gc_{gs[b]}")
                nc.scalar.copy(Gc, G)
                if gs[b] == 0:
                    hook("gc", Gc.bitcast(F32)[:, :])
                Gcs.append(Gc)

                        # trace: diag-mask partial sums (DVE) -> reduce + broadcast on PE
            # (via the widened s-bank: tr at [0, 256+b], bc at [:, 260+b])
            rcpbs, rcp2bs = [], []
            for b in range(W):
                g = gs[b]
                scr = small_p.tile([128, 128], F32, tag="scr", name=f"scr_{g}")
                dg = small_p.tile([128, 2], F32, tag="dg", name=f"dg_{g}")
                for m in range(2):
                    nc.vector.scalar_tensor_tensor(
                        scr, Gcs[b].bitcast(F32)[:, m * D + m * 128: m * D + (m + 1) * 128],
                        1.0, I128, ALU.mult, ALU.mult, accum_out=dg[:, m:m + 1])
                dgs = small_p.tile([128, 1], F32, tag="dgs", name=f"dgs_{g}")
                nc.vector.tensor_add(dgs, dg[:, 0:1], dg[:, 1:2])
                nc.tensor.matmul(s_ps[:, 260 + 4 * b:261 + 4 * b], ones128, dgs,
                                 start=True, stop=True)
                bc = s_ps[:, 260 + 4 * b:261 + 4 * b]
                rcpb = small_p.tile([128, 1], F32, tag="rcpb", name=f"rcpb_{g}")
                nc.vector.reciprocal(rcpb, bc)
                rcp2b = small_p.tile([128, 1], F32, tag="rcp2b", name=f"rcp2b_{g}")
                nc.vector.tensor_mul(rcp2b, rcpb, rcpb)
                # cb = sqrt(trc/(n-1)) * 0.03125/n, broadcast over partitions
                sq = small_p.tile([128, 1], F32, tag="sq", name=f"sq_{g}")
                nc.scalar.activation(sq, bc, ACTF.Sqrt, scale=1.0 / (NPG - 1))
                cb = small_p.tile([128, 1], F32, tag="cb", name=f"cb_{g}")
                nc.vector.tensor_scalar_mul(cb, sq, 0.03125 / NPG)
                rcpbs.append(rcpb)
                rcp2bs.append(rcp2b)
                mats[b]["cb"] = cb

            for b in range(W):
                A = mats_p.tile([128, 512], F32R, tag="A", name=f"A_{gs[b]}")
                nc.vector.tensor_scalar_mul(A, Gcs[b].bitcast(F32), rcpbs[b])
                if gs[b] == 0:
                    hook("a", A.bitcast(F32)[:, :])
                As.append(A)
                mats[b]["A"] = A

            # A2 (normalized via rcp^2 at evac)
            sta = [psStage.tile([128, 512], F32, tag="st", name=f"a2_{gs[b]}")
                   for b in range(W)]
            for b in range(W):
                mm256(sta[b], Gcs[b], Gcs[b])
            A2ns, W1s, V0s = [], [], []
            for b in range(W):
                A2n = chain_p.tile([128, 512], F32, tag="A2n", name=f"A2n_{gs[b]}")
                nc.scalar.activation(A2n, sta[b], ACTF.Copy, scale=rcp2bs[b])
                A2ns.append(A2n)
            for b in range(W):
                W1 = chain_p.tile([128, 512], F32R, tag="W1", name=f"W1_{gs[b]}")
                nc.vector.scalar_tensor_tensor(W1, As[b], 3.0, A2ns[b],
                                               ALU.mult, ALU.subtract)
                W1s.append(W1)
                V0 = chain_p.tile([128, 512], F32R, tag="V0", name=f"V0_{gs[b]}")
                nc.vector.scalar_tensor_tensor(V0, As[b], -1.0, cst3I,
                                               ALU.mult, ALU.add)
                V0s.append(V0)

            stp = [psStage.tile([128, 512], F32, tag="st", name=f"p_{gs[b]}")
                   for b in range(W)]
            T1s, V1s = [], []
            for b in range(W):
                mm256(stp[b], W1s[b], V0s[b])
            for b in range(W):
                T1 = mats_p.tile([128, 512], F32R, tag="T1", name=f"T1_{gs[b]}")
                nc.scalar.mul(T1, stp[b], 0.25)
                T1s.append(T1)
                mats[b]["T1"] = T1
            for b in range(W):
                V1 = chain_p.tile([128, 512], F32R, tag="V1", name=f"V1_{gs[b]}")
                nc.vector.scalar_tensor_tensor(V1, T1s[b], -1.0, cst3I,
                                               ALU.mult, ALU.add)
                V1s.append(V1)

            stq = [psStage.tile([128, 512], F32, tag="st", name=f"q_{gs[b]}")
                   for b in range(W)]
            Qbs = []
            for b in range(W):
                mm256(stq[b], T1s[b], V1s[b])
            for b in range(W):
                Qb = chain_p.tile([128, 512], F32R, tag="Qb", name=f"Qb_{gs[b]}")
                nc.scalar.copy(Qb, stq[b])
                Qbs.append(Qb)

            str_ = [psStage.tile([128, 512], F32, tag="st", name=f"rr_{gs[b]}")
                    for b in range(W)]
            T2s, V2s = [], []
            for b in range(W):
                mm256(str_[b], Qbs[b], V1s[b])
            for b in range(W):
                T2 = mats_p.tile([128, 512], F32R, tag="T2", name=f"T2_{gs[b]}")
                nc.scalar.mul(T2, str_[b], 0.25)
                T2s.append(T2)
                mats[b]["T2"] = T2
            for b in range(W):
                V2 = chain_p.tile([128, 512], F32R, tag="V2", name=f"V2_{gs[b]}")
                nc.vector.scalar_tensor_tensor(V2, T2s[b], -1.0, cst3I,
                                               ALU.mult, ALU.add)
                V2s.append(V2)

            sts5 = [psStage.tile([128, 512], F32, tag="st", name=f"s5_{gs[b]}")
                    for b in range(W)]
            Sbs = []
            for b in range(W):
                mm256(sts5[b], T2s[b], V2s[b])
            for b in range(W):
                Sb = chain_p.tile([128, 512], F32R, tag="Sb", name=f"Sb_{gs[b]}")
                nc.vector.tensor_copy(Sb, sts5[b])
                Sbs.append(Sb)

            stu = [psStage.tile([128, 512], F32, tag="st", name=f"u_{gs[b]}")
                   for b in range(W)]
            for b in range(W):
                mm256(stu[b], Sbs[b], V2s[b])
            for b in range(W):
                T3 = mats_p.tile([128, 512], F32R, tag="T3", name=f"T3_{gs[b]}")
                nc.scalar.mul(T3, stu[b], 0.25)
                if gs[b] == 0:
                    hook("t3", T3.bitcast(F32)[:, :])
                mats[b]["T3"] = T3
            return mats

        def tail_step(si, wave, cur, mats, kind, v0c3, tailidx):
            """One tail step for all W graphs.

            rows land at 32-aligned PSUM partitions, transposed back to
            column form [128, 2W] with the M32 selector."""
            xkey = {0: "T3", 1: "T3", 2: "T3", 3: "T3",
                    4: "T2", 5: "T1", 6: "A", 7: "A"}[si]
            # f32r matmuls require dst partition base 0: each graph's row
            # lands as a [W,256] block at partitions 0:W (junk rows = other
            # cur columns vs X_b), two graphs per PSUM bank.
            rows = [psRows.tile([W, 512], F32, tag=f"rows{h}",
                                name=f"rows{h}_{wave}_{si}") for h in range(2)]
            for b in range(W):
                X = mats[b][xkey]
                dst = rows[b // 2][:, (b % 2) * D:(b % 2 + 1) * D]
                for k in range(2):
                    nc.tensor.matmul(dst, cur[:, k * W:(k + 1) * W],
                                     X[:, k * D:(k + 1) * D],
                                     start=(k == 0), stop=(k == 1))
            if kind == "final":
                for h in range(2):
                    osb = tail_p.tile([W, 512], F32, tag=f"osb{h}",
                                      name=f"osb{h}_{wave}")
                    nc.scalar.copy(osb, rows[h])
                    for j in range(2):
                        b = 2 * h + j
                        nc.sync.dma_start(out_d[wave * W + b:wave * W + b + 1, :],
                                          osb[b:b + 1, j * D:(j + 1) * D])
                return None
            usb = []
            for h in range(2):
                u = tail_p.tile([W, 512], F32R, tag=f"usb{h}",
                                name=f"usb{h}_{wave}_{si}")
                if h == 0:
                    nc.scalar.copy(u, rows[h])
                else:
                    nc.vector.tensor_copy(u, rows[h])
                usb.append(u)
            tpc = psTpc.tile([128, 40], F32, tag="tpc", name=f"tpc_{wave}_{si}")
            for b in range(W):
                for m in range(2):
                    nc.tensor.matmul(tpc[:, (m * W + b) * 4:(m * W + b + 1) * 4],
                                     usb[b // 2][0:W, (b % 2) * D + m * 128:
                                                 (b % 2) * D + (m + 1) * 128],
                                     E4SEL[0:W, 4 * b:4 * (b + 1)],
                                     start=True, stop=True)
            ucs = tpc[:, 0:32].rearrange("p (c j) -> p c j", j=4)[:, :, 0]
            nxt = tail_p.tile([128, 2 * W], F32R, tag="cur",
                              name=f"cur_{wave}_{si}")
            if kind == "comb":
                nc.vector.scalar_tensor_tensor(nxt, cur, 3.0, ucs,
                                               ALU.mult, ALU.subtract)
            elif kind == "a3":
                nc.vector.scalar_tensor_tensor(nxt, ucs, -0.25, v0c3,
                                               ALU.mult, ALU.add)
            return nxt

        for wave in range(N_WAVES):
            s_ps = psS.tile([128, 276], F32, tag="s", name=f"s_{wave}")
            if n_zeroed["s"] < 1:
                n_zeroed["s"] += 1
                nc.scalar.memzero(s_ps)
            gbs = []
            for b in range(W):
                g = wave * W + b
                gbs.append(phase_a1(g, s_ps, b))
            load_cst()
            # s evac + bf16 row tiles for the rank-1 update
            S4 = wave_p.tile([128, 256], F32R, tag="S4", name=f"S4_{wave}")
            nc.scalar.copy(S4, s_ps[:, 0:256])
            SB4 = wave_p.tile([128, 256], BF16, tag="SB4", name=f"SB4_{wave}")
            nc.scalar.copy(SB4, s_ps[:, 0:256])
            SBn4 = wave_p.tile([128, 256], BF16, tag="SBn4", name=f"SBn4_{wave}")
            nc.vector.tensor_scalar_mul(SBn4, SB4, -1.0 / NPG)
            sts = []
            for b in range(W):
                g = wave * W + b
                sts.append(phase_a2(g, gbs[b], SB4, SBn4, b))
            mats = phase_b(wave, sts, s_ps, S4, SB4, SBn4)

            # v0 columns via cb-valued selector
            E = wave_p.tile([128, W], F32R, tag="E", name=f"E_{wave}")
            for b in range(W):
                nc.vector.scalar_tensor_tensor(E[:, b:b + 1], mats[b]["cb"], 1.0,
                                               M32f[:, b:b + 1], ALU.mult, ALU.mult)
            tpv = psTpc.tile([128, 40], F32, tag="tpc", name=f"tpv_{wave}")
            for m in range(2):
                nc.tensor.matmul(tpv[:, 32 + m * W:32 + (m + 1) * W],
                                 S4[:, m * 128:(m + 1) * 128],
                                 E, start=True, stop=True)
            v0c = tail_p.tile([128, 2 * W], F32R, tag="cur", name=f"v0c_{wave}")
            nc.scalar.copy(v0c, tpv[:, 32:40])
            v0c3 = tail_p.tile([128, 2 * W], F32R, tag="v0c3", name=f"v0c3_{wave}")
            nc.vector.tensor_scalar_mul(v0c3, v0c, 3.0)

            if wave == 0:
                hook("s", s_ps[:, :])
                hook("v0", v0c.bitcast(F32)[:, :])
            cur = v0c
            kinds = ["comb", "comb", "a3", "comb", "comb", "comb", "comb", "final"]
            for si in range(8):
                cur = tail_step(si, wave, cur, mats, kinds[si], v0c3, si)
                if wave == 0 and si == 0:
                    hook("c1", cur.bitcast(F32)[:, :])


_CACHED_NC = None


def _get_nc():
    global _CACHED_NC
    if _CACHED_NC is None:
        _CACHED_NC = build_module()
    return _CACHED_NC


def _run(feat, noise, **spmd_kwargs):
    feat = np.ascontiguousarray(np.asarray(feat), dtype=np.float32)
    noise01 = np.asarray(noise, dtype=np.float32) * np.float32(0.01)
    noise01 = np.ascontiguousarray(noise01)
    cst, cstb = _const_arrays()
    nc = _get_nc()
    in_maps = []
    for c in range(N_CORES):
        in_maps.append({
            "feat": feat[c * ROWS_CORE:(c + 1) * ROWS_CORE],
            "noise": noise01[c * ROWS_CORE:(c + 1) * ROWS_CORE],
            "cst": cst,
            "cstb": cstb,
        })
    return run_bass_kernel_spmd(nc, in_maps, list(range(N_CORES)), **spmd_kwargs)


def kernel(feat, noise, n_per_graph):
    assert int(n_per_graph) == NPG
    try:
        res = _run(feat, noise)
    except Exception:
        # the axon device occasionally reports a transient unrecoverable
        # state; one retry usually succeeds
        res = _run(feat, noise)
    return np.concatenate([res.results[c]["out"] for c in range(N_CORES)], axis=0)
